# revision 6
# baseline (speedup 1.0000x reference)
"""Trainium2 Bass kernel: GatedRecurrentCell (v2).

Math (per batch b, channel i, time t):
    pa = x @ Wa^T + ba ; pi = x @ Wi^T + bi
    a  = sigmoid(gate) * 3**(-sigmoid(pa))
    c  = sqrt(1-a^2) * silu(pi + bi)
    h_t = a_t*h_{t-1} + c_t   (h_{-1} = 0);  out = h

Key trick: 3**(-sigmoid(p)) == FA - FB*tanh(FC*p + FD) to 5.5e-4 abs
(global least-squares fit), so  a = aA - aB*tanh(FC*pa + tb)  with
per-channel aA = sigmoid(gate)*FA, aB = sigmoid(gate)*FB, tb = FC*ba+FD.
This removes the Exp ACT pass and its table set entirely: the scalar
engine only runs {Silu,Tanh} (silu_and_others) and {Sqrt}
(sqrt_and_others) - 2 table loads per chunk-group instead of 3, and the
exp pass budget moves to a cheap tensor_scalar affine on GPSIMD.

Mapping: data-parallel over batch (8 cores, 1 batch each); channels on
partitions (16 chunks of 128), time on the free dim. GEMMs in bf16
(k-outer per supertile; FWL applies to bf16 weights). The recurrence
runs as independent per-chunk tensor_tensor_scan ops split across DVE
and GPSIMD. h is written back in bf16 and upcast on the host.
"""

import functools
import os

import numpy as np

B, S, D, I = 8, 2048, 512, 2048
P = 128
NCORES = 8

# fit of 3^(-sigmoid(p)) = FA - FB*tanh(FC*p + FD), max abs err 5.5e-4
FA = 0.66661083
FB = 0.33324857
FC = 0.5096609
FD = 0.27426951

# knobs
G = int(os.environ.get("GRC_G", "3"))            # chunks per ACT table group
# tensor_tensor_scan is NOT a valid POOL opcode (walrus ISA check) -> DVE only
SCAN_GP = os.environ.get("GRC_SCAN_GP", "none")
AFF_ENG = os.environ.get("GRC_AFF", "gp")        # gp|dve  (a = nB*t + aA)
A2_ENG = os.environ.get("GRC_A2", "gp")          # gp|dve  (a2 = a*a)
CW = 1024                                        # PSUM supertile width


def _build_nc(s, d, i, silu=True):
    import concourse.bacc as bacc
    import concourse.mybir as mybir
    import concourse.tile as tile
    from concourse.tile import add_dep_helper
    from contextlib import ExitStack

    F32 = mybir.dt.float32
    BF16 = mybir.dt.bfloat16
    AF = mybir.ActivationFunctionType
    ALU = mybir.AluOpType

    nd = d // P            # contraction chunks
    ni = i // P            # channel chunks
    cw = min(CW, s)
    nh = s // cw           # supertiles per channel row
    nmm = cw // 512        # matmuls (N=512) per supertile

    nc = bacc.Bacc("TRN2", target_bir_lowering=False, debug=False,
                   num_devices=NCORES)

    xT_d = nc.dram_tensor("xT", [d, s], BF16, kind="ExternalInput").ap()
    waT_d = nc.dram_tensor("WaT", [ni, P, d], BF16, kind="ExternalInput").ap()
    wiT_d = nc.dram_tensor("WiT", [ni, P, d], BF16, kind="ExternalInput").ap()
    aA_d = nc.dram_tensor("aA", [P, ni], F32, kind="ExternalInput").ap()
    nB_d = nc.dram_tensor("nB", [P, ni], F32, kind="ExternalInput").ap()
    tb_d = nc.dram_tensor("tb", [P, ni], F32, kind="ExternalInput").ap()
    sb_d = nc.dram_tensor("sb", [P, ni], F32, kind="ExternalInput").ap()
    out_d = nc.dram_tensor("out", [i, s], BF16, kind="ExternalOutput").ap()

    with tile.TileContext(nc) as tc:
        with ExitStack() as ctx:
            const_pool = ctx.enter_context(tc.tile_pool(name="const", bufs=1))
            xt_pool = ctx.enter_context(tc.tile_pool(name="xt", bufs=1))
            wst_pool = ctx.enter_context(tc.tile_pool(name="wst", bufs=1))
            ps_pool = ctx.enter_context(
                tc.tile_pool(name="mmpsum", bufs=1, space="PSUM"))
            rows = ctx.enter_context(tc.tile_pool(name="rows", bufs=1))

            aA_t = const_pool.tile([P, ni], F32, name="aA_t")
            nc.sync.dma_start(aA_t[:], aA_d[:])
            nB_t = const_pool.tile([P, ni], F32, name="nB_t")
            nc.sync.dma_start(nB_t[:], nB_d[:])
            tb_t = const_pool.tile([P, ni], F32, name="tb_t")
            nc.sync.dma_start(tb_t[:], tb_d[:])
            sb_t = const_pool.tile([P, ni], F32, name="sb_t")
            nc.sync.dma_start(sb_t[:], sb_d[:])

            # resident x^T, column-chunked k-interleaved loads
            xT_sb = [xt_pool.tile([P, s], BF16, name=f"xT{k}") for k in
                     range(nd)]
            for h in range(nh):
                for k in range(nd):
                    nc.sync.dma_start(
                        xT_sb[k][:, h * cw:(h + 1) * cw],
                        xT_d[k * P:(k + 1) * P, h * cw:(h + 1) * cw])

            act_chain = []

            def act(out_ap, in_ap, func, **kw):
                inst = nc.scalar.activation(out_ap, in_ap, func, **kw)
                if act_chain:
                    add_dep_helper(inst.ins, act_chain[-1].ins, False,
                                   "act table phase order")
                act_chain.append(inst)
                return inst

            def gemm(ps, w_sb, h):
                # k-outer: consecutive matmuls share lhsT
                for k in range(nd):
                    for m in range(nmm):
                        lo = h * cw + m * 512
                        nc.tensor.matmul(
                            ps[:, m * 512:(m + 1) * 512],
                            w_sb[:, k * P:(k + 1) * P],
                            xT_sb[k][:, lo:lo + 512],
                            start=(k == 0), stop=(k == nd - 1))

            groups = [list(range(g0, min(g0 + G, ni)))
                      for g0 in range(0, ni, G)]

            state = {}   # ic -> dict of tiles

            def phase_silu_tanh(ics):
                for ic in ics:
                    wi_sb = wst_pool.tile([P, d], BF16, name=f"wi{ic}",
                                          tag="wi", bufs=3)
                    nc.sync.dma_start(wi_sb[:], wiT_d[ic])
                    wa_sb = wst_pool.tile([P, d], BF16, name=f"wa{ic}",
                                          tag="wa", bufs=3)
                    nc.sync.dma_start(wa_sb[:], waT_d[ic])

                    w_t = rows.tile([P, s], BF16, name=f"w{ic}", tag="w",
                                    bufs=8)
                    a_t = rows.tile([P, s], F32, name=f"a{ic}", tag="a",
                                    bufs=8)
                    a2_t = rows.tile([P, s], F32, name=f"a2{ic}", tag="a2",
                                     bufs=6)
                    st = {"w": w_t, "a": a_t, "a2": a2_t}
                    state[ic] = st

                    for h in range(nh):
                        sl = slice(h * cw, (h + 1) * cw)
                        pi_ps = ps_pool.tile([P, cw], F32, name=f"pi{ic}_{h}",
                                             tag="pi", bufs=2)
                        gemm(pi_ps, wi_sb, h)
                        if silu:
                            act(w_t[:, sl], pi_ps[:], AF.Silu,
                                bias=sb_t[:, ic:ic + 1])
                        else:
                            # CoreSim fallback: silu = sigmoid(z)*z
                            sg = rows.tile([P, cw], F32, name=f"sg{ic}_{h}",
                                           tag="sg", bufs=3)
                            act(sg[:], pi_ps[:], AF.Sigmoid,
                                bias=sb_t[:, ic:ic + 1])
                            pib = rows.tile([P, cw], F32, name=f"pib{ic}_{h}",
                                            tag="pib", bufs=3)
                            act(pib[:], pi_ps[:], AF.Identity,
                                bias=sb_t[:, ic:ic + 1])
                            nc.vector.tensor_mul(w_t[:, sl], sg[:], pib[:])
                    for h in range(nh):
                        sl = slice(h * cw, (h + 1) * cw)
                        pa_ps = ps_pool.tile([P, cw], F32, name=f"pa{ic}_{h}",
                                             tag="pa", bufs=2)
                        gemm(pa_ps, wa_sb, h)
                        # a_t first holds t = tanh(FC*pa + tb)
                        act(a_t[:, sl], pa_ps[:], AF.Tanh,
                            scale=FC, bias=tb_t[:, ic:ic + 1])
                    # a = nB*t + aA  (in-place), then a2 = a*a (DVE)
                    aff_eng = nc.gpsimd if AFF_ENG == "gp" else nc.vector
                    aff_eng.tensor_scalar(
                        a_t[:], a_t[:], nB_t[:, ic:ic + 1],
                        aA_t[:, ic:ic + 1], op0=ALU.mult, op1=ALU.add)
                    a2_eng = nc.gpsimd if A2_ENG == "gp" else nc.vector
                    a2_eng.tensor_mul(a2_t[:], a_t[:], a_t[:])

            def phase_sqrt_scan(ics):
                for ic in ics:
                    st = state.pop(ic)
                    q_t = rows.tile([P, s], BF16, name=f"q{ic}", tag="q",
                                    bufs=3)
                    act(q_t[:], st["a2"][:], AF.Sqrt, scale=-1.0, bias=1.0)
                    # c = q*w in place over w
                    nc.vector.tensor_mul(st["w"][:], q_t[:], st["w"][:])
                    h_t = rows.tile([P, s], BF16, name=f"h{ic}", tag="h",
                                    bufs=3)
                    use_gp = (SCAN_GP == "all" or
                              (SCAN_GP == "odd" and ic % 2 == 1))
                    eng = nc.gpsimd if use_gp else nc.vector
                    eng.tensor_tensor_scan(
                        h_t[:], st["a"][:], st["w"][:], 0.0,
                        op0=ALU.mult, op1=ALU.add)
                    nc.sync.dma_start(out_d[ic * P:(ic + 1) * P, :], h_t[:])

            prev = None
            for ics in groups:
                phase_silu_tanh(ics)
                if prev is not None:
                    phase_sqrt_scan(prev)
                prev = ics
            phase_sqrt_scan(prev)

    nc.compile()
    return nc


@functools.lru_cache(maxsize=2)
def _get_nc(s=S, d=D, i=I):
    return _build_nc(s, d, i)


LAST_RESULTS = None


def _prep_core_inputs(xb, shared):
    import ml_dtypes
    xT = np.ascontiguousarray(xb.T).astype(ml_dtypes.bfloat16)
    m = {"xT": xT}
    m.update(shared)
    return m


def _prep_shared(Wa, ba, Wi, bi, gate, d, i):
    import ml_dtypes
    ni = i // P
    nd = d // P
    # WaT[ic, p, k*128+j] = Wa[ic*128+j, k*128+p]  (lhsT blocks)
    WaT = np.ascontiguousarray(
        Wa.reshape(ni, P, nd, P).transpose(0, 3, 2, 1).reshape(ni, P, d)
    ).astype(ml_dtypes.bfloat16)
    WiT = np.ascontiguousarray(
        Wi.reshape(ni, P, nd, P).transpose(0, 3, 2, 1).reshape(ni, P, d)
    ).astype(ml_dtypes.bfloat16)
    alpha = 1.0 / (1.0 + np.exp(-gate.astype(np.float64)))
    aA = np.ascontiguousarray((alpha * FA).astype(np.float32).reshape(ni, P).T)
    nB = np.ascontiguousarray((-alpha * FB).astype(np.float32).reshape(ni, P).T)
    tb = np.ascontiguousarray(
        (FC * ba.astype(np.float64) + FD).astype(np.float32).reshape(ni, P).T)
    sb = np.ascontiguousarray(bi.astype(np.float32).reshape(ni, P).T)
    return {"WaT": WaT, "WiT": WiT, "aA": aA, "nB": nB, "tb": tb, "sb": sb}


def kernel(x, Wa, ba, Wi, bi, gate):
    global LAST_RESULTS
    from concourse.bass_utils import run_bass_kernel_spmd

    x = np.asarray(x, dtype=np.float32)
    b, s, d = x.shape
    i = Wa.shape[0]
    nc = _get_nc(s, d, i)

    shared = _prep_shared(
        np.asarray(Wa, np.float32), np.asarray(ba, np.float32),
        np.asarray(Wi, np.float32), np.asarray(bi, np.float32),
        np.asarray(gate, np.float32), d, i)

    in_maps = [_prep_core_inputs(x[bb], shared) for bb in range(b)]
    res = run_bass_kernel_spmd(nc, in_maps, list(range(b)))
    LAST_RESULTS = res
    out = np.stack([np.asarray(res.results[bb]["out"]).astype(np.float32).T
                    for bb in range(b)], axis=0)
    return np.ascontiguousarray(out)


# revision 10
# speedup vs baseline: 1.0253x; 1.0253x over previous
"""Trainium2 Bass kernel: GatedRecurrentCell (v2).

Math (per batch b, channel i, time t):
    pa = x @ Wa^T + ba ; pi = x @ Wi^T + bi
    a  = sigmoid(gate) * 3**(-sigmoid(pa))
    c  = sqrt(1-a^2) * silu(pi + bi)
    h_t = a_t*h_{t-1} + c_t   (h_{-1} = 0);  out = h

Key trick: 3**(-sigmoid(p)) == FA - FB*tanh(FC*p + FD) to 5.5e-4 abs
(global least-squares fit), so  a = aA - aB*tanh(FC*pa + tb)  with
per-channel aA = sigmoid(gate)*FA, aB = sigmoid(gate)*FB, tb = FC*ba+FD.
This removes the Exp ACT pass and its table set entirely: the scalar
engine only runs {Silu,Tanh} (silu_and_others) and {Sqrt}
(sqrt_and_others) - 2 table loads per chunk-group instead of 3, and the
exp pass budget moves to a cheap tensor_scalar affine on GPSIMD.

Mapping: data-parallel over batch (8 cores, 1 batch each); channels on
partitions (16 chunks of 128), time on the free dim. GEMMs in bf16
(k-outer per supertile; FWL applies to bf16 weights). The recurrence
runs as independent per-chunk tensor_tensor_scan ops split across DVE
and GPSIMD. h is written back in bf16 and upcast on the host.
"""

import functools
import os

import numpy as np

B, S, D, I = 8, 2048, 512, 2048
P = 128
NCORES = 8

# fit of 3^(-sigmoid(p)) = FA - FB*tanh(FC*p + FD), max abs err 5.5e-4
FA = 0.66661083
FB = 0.33324857
FC = 0.5096609
FD = 0.27426951

# knobs
G = int(os.environ.get("GRC_G", "3"))            # chunks per ACT table group
# tensor_tensor_scan is NOT a valid POOL opcode (walrus ISA check) -> DVE only
SCAN_GP = os.environ.get("GRC_SCAN_GP", "none")
AFF_ENG = os.environ.get("GRC_AFF", "dve")       # gp|dve  (a = nB*t + aA)
A2_ENG = os.environ.get("GRC_A2", "gp")          # gp|dve  (a2 = a*a)
CMUL_ENG = os.environ.get("GRC_CMUL", "dve")     # gp|dve  (c = q*w)
CW = 1024                                        # PSUM supertile width


def _build_nc(s, d, i, silu=True):
    import concourse.bacc as bacc
    import concourse.mybir as mybir
    import concourse.tile as tile
    from concourse.tile import add_dep_helper
    from contextlib import ExitStack

    F32 = mybir.dt.float32
    BF16 = mybir.dt.bfloat16
    AF = mybir.ActivationFunctionType
    ALU = mybir.AluOpType

    nd = d // P            # contraction chunks
    ni = i // P            # channel chunks
    cw = min(CW, s)
    nh = s // cw           # supertiles per channel row
    nmm = cw // 512        # matmuls (N=512) per supertile

    nc = bacc.Bacc("TRN2", target_bir_lowering=False, debug=False,
                   num_devices=NCORES)

    xT_d = nc.dram_tensor("xT", [d, s], BF16, kind="ExternalInput").ap()
    waT_d = nc.dram_tensor("WaT", [ni, P, d], BF16, kind="ExternalInput").ap()
    wiT_d = nc.dram_tensor("WiT", [ni, P, d], BF16, kind="ExternalInput").ap()
    aA_d = nc.dram_tensor("aA", [P, ni], F32, kind="ExternalInput").ap()
    nB_d = nc.dram_tensor("nB", [P, ni], F32, kind="ExternalInput").ap()
    tb_d = nc.dram_tensor("tb", [P, ni], F32, kind="ExternalInput").ap()
    sb_d = nc.dram_tensor("sb", [P, ni], F32, kind="ExternalInput").ap()
    out_d = nc.dram_tensor("out", [i, s], BF16, kind="ExternalOutput").ap()

    with tile.TileContext(nc) as tc:
        with ExitStack() as ctx:
            const_pool = ctx.enter_context(tc.tile_pool(name="const", bufs=1))
            xt_pool = ctx.enter_context(tc.tile_pool(name="xt", bufs=1))
            wst_pool = ctx.enter_context(tc.tile_pool(name="wst", bufs=1))
            ps_pool = ctx.enter_context(
                tc.tile_pool(name="mmpsum", bufs=1, space="PSUM"))
            rows = ctx.enter_context(tc.tile_pool(name="rows", bufs=1))

            aA_t = const_pool.tile([P, ni], F32, name="aA_t")
            nc.sync.dma_start(aA_t[:], aA_d[:])
            nB_t = const_pool.tile([P, ni], F32, name="nB_t")
            nc.sync.dma_start(nB_t[:], nB_d[:])
            tb_t = const_pool.tile([P, ni], F32, name="tb_t")
            nc.sync.dma_start(tb_t[:], tb_d[:])
            sb_t = const_pool.tile([P, ni], F32, name="sb_t")
            nc.sync.dma_start(sb_t[:], sb_d[:])

            # resident x^T, column-chunked k-interleaved loads
            xT_sb = [xt_pool.tile([P, s], BF16, name=f"xT{k}") for k in
                     range(nd)]
            for h in range(nh):
                for k in range(nd):
                    nc.sync.dma_start(
                        xT_sb[k][:, h * cw:(h + 1) * cw],
                        xT_d[k * P:(k + 1) * P, h * cw:(h + 1) * cw])

            act_chain = []

            def act(out_ap, in_ap, func, **kw):
                inst = nc.scalar.activation(out_ap, in_ap, func, **kw)
                if act_chain:
                    add_dep_helper(inst.ins, act_chain[-1].ins, False,
                                   "act table phase order")
                act_chain.append(inst)
                return inst

            def gemm(ps, w_sb, h):
                # k-outer: consecutive matmuls share lhsT
                for k in range(nd):
                    for m in range(nmm):
                        lo = h * cw + m * 512
                        nc.tensor.matmul(
                            ps[:, m * 512:(m + 1) * 512],
                            w_sb[:, k * P:(k + 1) * P],
                            xT_sb[k][:, lo:lo + 512],
                            start=(k == 0), stop=(k == nd - 1))

            groups = [list(range(g0, min(g0 + G, ni)))
                      for g0 in range(0, ni, G)]

            state = {}   # ic -> dict of tiles

            def phase_silu_tanh(ics):
                for ic in ics:
                    wi_sb = wst_pool.tile([P, d], BF16, name=f"wi{ic}",
                                          tag="wi", bufs=3)
                    nc.sync.dma_start(wi_sb[:], wiT_d[ic])
                    wa_sb = wst_pool.tile([P, d], BF16, name=f"wa{ic}",
                                          tag="wa", bufs=3)
                    nc.sync.dma_start(wa_sb[:], waT_d[ic])

                    w_t = rows.tile([P, s], BF16, name=f"w{ic}", tag="w",
                                    bufs=7)
                    th_t = rows.tile([P, s], F32, name=f"th{ic}", tag="th",
                                     bufs=3)
                    a_t = rows.tile([P, s], F32, name=f"a{ic}", tag="a",
                                    bufs=7)
                    a2_t = rows.tile([P, s], F32, name=f"a2{ic}", tag="a2",
                                     bufs=5)
                    st = {"w": w_t, "a": a_t, "a2": a2_t}
                    state[ic] = st

                    for h in range(nh):
                        sl = slice(h * cw, (h + 1) * cw)
                        pi_ps = ps_pool.tile([P, cw], F32, name=f"pi{ic}_{h}",
                                             tag="pi", bufs=2)
                        gemm(pi_ps, wi_sb, h)
                        if silu:
                            act(w_t[:, sl], pi_ps[:], AF.Silu,
                                bias=sb_t[:, ic:ic + 1])
                        else:
                            # CoreSim fallback: silu = sigmoid(z)*z
                            sg = rows.tile([P, cw], F32, name=f"sg{ic}_{h}",
                                           tag="sg", bufs=3)
                            act(sg[:], pi_ps[:], AF.Sigmoid,
                                bias=sb_t[:, ic:ic + 1])
                            pib = rows.tile([P, cw], F32, name=f"pib{ic}_{h}",
                                            tag="pib", bufs=3)
                            act(pib[:], pi_ps[:], AF.Identity,
                                bias=sb_t[:, ic:ic + 1])
                            nc.vector.tensor_mul(w_t[:, sl], sg[:], pib[:])
                    for h in range(nh):
                        sl = slice(h * cw, (h + 1) * cw)
                        pa_ps = ps_pool.tile([P, cw], F32, name=f"pa{ic}_{h}",
                                             tag="pa", bufs=2)
                        gemm(pa_ps, wa_sb, h)
                        act(th_t[:, sl], pa_ps[:], AF.Tanh,
                            scale=FC, bias=tb_t[:, ic:ic + 1])
                    # a = nB*th + aA, then a2 = a*a
                    aff_eng = nc.gpsimd if AFF_ENG == "gp" else nc.vector
                    aff_eng.tensor_scalar(
                        a_t[:], th_t[:], nB_t[:, ic:ic + 1],
                        aA_t[:, ic:ic + 1], op0=ALU.mult, op1=ALU.add)
                    a2_eng = nc.gpsimd if A2_ENG == "gp" else nc.vector
                    a2_eng.tensor_mul(a2_t[:], a_t[:], a_t[:])

            def phase_sqrt_scan(ics):
                for ic in ics:
                    st = state.pop(ic)
                    q_t = rows.tile([P, s], BF16, name=f"q{ic}", tag="q",
                                    bufs=3)
                    act(q_t[:], st["a2"][:], AF.Sqrt, scale=-1.0, bias=1.0)
                    c_t = rows.tile([P, s], BF16, name=f"c{ic}", tag="c",
                                    bufs=3)
                    cm_eng = nc.gpsimd if CMUL_ENG == "gp" else nc.vector
                    cm_eng.tensor_mul(c_t[:], q_t[:], st["w"][:])
                    h_t = rows.tile([P, s], BF16, name=f"h{ic}", tag="h",
                                    bufs=3)
                    use_gp = (SCAN_GP == "all" or
                              (SCAN_GP == "odd" and ic % 2 == 1))
                    eng = nc.gpsimd if use_gp else nc.vector
                    eng.tensor_tensor_scan(
                        h_t[:], st["a"][:], c_t[:], 0.0,
                        op0=ALU.mult, op1=ALU.add)
                    nc.sync.dma_start(out_d[ic * P:(ic + 1) * P, :], h_t[:])

            prev = None
            for ics in groups:
                phase_silu_tanh(ics)
                if prev is not None:
                    phase_sqrt_scan(prev)
                prev = ics
            phase_sqrt_scan(prev)

    nc.compile()
    return nc


@functools.lru_cache(maxsize=2)
def _get_nc(s=S, d=D, i=I):
    return _build_nc(s, d, i)


LAST_RESULTS = None


def _prep_core_inputs(xb, shared):
    import ml_dtypes
    xT = np.ascontiguousarray(xb.T).astype(ml_dtypes.bfloat16)
    m = {"xT": xT}
    m.update(shared)
    return m


def _prep_shared(Wa, ba, Wi, bi, gate, d, i):
    import ml_dtypes
    ni = i // P
    nd = d // P
    # WaT[ic, p, k*128+j] = Wa[ic*128+j, k*128+p]  (lhsT blocks)
    WaT = np.ascontiguousarray(
        Wa.reshape(ni, P, nd, P).transpose(0, 3, 2, 1).reshape(ni, P, d)
    ).astype(ml_dtypes.bfloat16)
    WiT = np.ascontiguousarray(
        Wi.reshape(ni, P, nd, P).transpose(0, 3, 2, 1).reshape(ni, P, d)
    ).astype(ml_dtypes.bfloat16)
    alpha = 1.0 / (1.0 + np.exp(-gate.astype(np.float64)))
    aA = np.ascontiguousarray((alpha * FA).astype(np.float32).reshape(ni, P).T)
    nB = np.ascontiguousarray((-alpha * FB).astype(np.float32).reshape(ni, P).T)
    tb = np.ascontiguousarray(
        (FC * ba.astype(np.float64) + FD).astype(np.float32).reshape(ni, P).T)
    sb = np.ascontiguousarray(bi.astype(np.float32).reshape(ni, P).T)
    return {"WaT": WaT, "WiT": WiT, "aA": aA, "nB": nB, "tb": tb, "sb": sb}


def kernel(x, Wa, ba, Wi, bi, gate):
    global LAST_RESULTS
    from concourse.bass_utils import run_bass_kernel_spmd

    x = np.asarray(x, dtype=np.float32)
    b, s, d = x.shape
    i = Wa.shape[0]
    nc = _get_nc(s, d, i)

    shared = _prep_shared(
        np.asarray(Wa, np.float32), np.asarray(ba, np.float32),
        np.asarray(Wi, np.float32), np.asarray(bi, np.float32),
        np.asarray(gate, np.float32), d, i)

    in_maps = [_prep_core_inputs(x[bb], shared) for bb in range(b)]
    res = run_bass_kernel_spmd(nc, in_maps, list(range(b)))
    LAST_RESULTS = res
    out = np.stack([np.asarray(res.results[bb]["out"]).astype(np.float32).T
                    for bb in range(b)], axis=0)
    return np.ascontiguousarray(out)
